# revision 1
# baseline (speedup 1.0000x reference)
"""GNN message-passing (Net3D) Trainium2 kernel, 8-way SPMD.

Strategy
--------
* Sort edges by destination node, shard them across 8 cores by contiguous
  dst ranges (so each core's partial segment-sums cover only its own node
  slice -> no all-reduce of [N,H] partials; a cheap AllGather of the
  updated feature slices replaces it).
* Node layout is padded per *graph* (each graph gets a fixed NG-slot
  block, 8 graphs per core) so per-graph readout reduces are uniform
  across cores (single SPMD program).
* Per 256-node window, edges are padded to a uniform chunk count KW so
  every core runs an identical instruction stream; ghost edges carry
  dst_local = -1 which zeroes their one-hot row in the segment-sum
  matmul.
* All activations live feature-major ([H, items]); feat[src]/feat[dst]
  row-gathers (dma_gather) are PE-transposed on chip. segment_sum is a
  PE matmul with a DVE-built (iota == dst_local) * edge_weight selection
  matrix. Layer 0 needs no gathers at all (feat0 is a broadcast
  embedding folded into biases on the host).
* Edge state d ([E,H]) streams through HBM in feature-major layout with
  ping-pong buffers between layers.
"""

import numpy as np

H = 128
G_FIX = 64
NCORES = 8
WIN = 256
F32MIN = -1.0e30


def _wrap_idxs(idx):
    """int idx [n] -> dma_gather layout [128, n/16] int16 (idx j at
    [j%16, j//16], replicated across the 8 groups of 16 partitions)."""
    n = idx.shape[0]
    assert n % 16 == 0
    t = idx.astype(np.int16).reshape(n // 16, 16).T  # [16, n/16]
    return np.tile(t, (8, 1))


def _prep(inputs):
    """Host-side graph preprocessing -> (params dict, per-core in_maps)."""
    d0 = np.ascontiguousarray(np.asarray(inputs["d"], dtype=np.float32))
    src = np.asarray(inputs["src"]).astype(np.int64)
    dst = np.asarray(inputs["dst"]).astype(np.int64)
    ngid = np.asarray(inputs["node_graph_id"]).astype(np.int64)
    G = int(np.asarray(inputs["num_graphs"]))
    N = ngid.shape[0]
    E = src.shape[0]
    assert G == G_FIX and G % NCORES == 0

    cnt = np.bincount(ngid, minlength=G).astype(np.int64)
    NG = int(max(-(-int(cnt.max()) // 32) * 32, 32))   # graph slot size
    NS = G * NG // NCORES                              # node slots per core
    Npad = G * NG
    NW = NS // WIN                                     # windows per core
    assert NS % WIN == 0 and Npad < 32768              # int16 gather idx

    gstart = np.zeros(G, np.int64)
    gstart[1:] = np.cumsum(cnt)[:-1]
    # node -> padded slot; real nodes spread evenly through each graph's
    # NG-slot block so ghost slots don't concentrate (equalizes per-window
    # edge counts -> smaller uniform chunk count KW)
    pos_in_g = np.arange(N) - gstart[ngid]
    padpos = ngid * NG + (pos_in_g * NG) // np.maximum(cnt[ngid], 1)
    psrc = padpos[src]
    pdst = padpos[dst]

    order = np.argsort(pdst, kind="stable")
    pdst_s = pdst[order]
    psrc_s = psrc[order]
    d0_s = d0[order]

    NWG = Npad // WIN                                  # global window count
    wstart = np.searchsorted(pdst_s, np.arange(NWG + 1) * WIN)
    wcnt = np.diff(wstart)
    KW = int(-(-int(wcnt.max()) // 128))
    KW = max(4, -(-KW // 4) * 4)                       # multiple of 4
    CAP = KW * 128
    Ecap = NW * CAP
    C = NW * KW                                        # chunks per core

    inv = 1.0 / np.maximum(cnt, 1)
    invcntR = np.tile(inv.astype(np.float32), (128, 1))
    presentR = np.tile((cnt > 0).astype(np.float32), (128, 1))

    # ---- weights / constants (shared by all cores) ----
    f32 = np.float32
    emb = np.asarray(inputs["node_embedding"], f32)            # [128]
    We = np.asarray(inputs["We"], f32)                         # [3,128]
    be = np.asarray(inputs["be"], f32)
    W1 = np.asarray(inputs["mpW1"], f32)                       # [4,384,128]
    b1 = np.asarray(inputs["mpb1"], f32)
    W2 = np.asarray(inputs["mpW2"], f32)
    b2 = np.asarray(inputs["mpb2"], f32)
    Ws = np.asarray(inputs["mpWs"], f32)                       # [4,128,1]
    bs = np.asarray(inputs["mpbs"], f32)                       # [4,1]
    Wu1 = np.asarray(inputs["mpWu1"], f32)
    bu1 = np.asarray(inputs["mpbu1"], f32)
    Wu2 = np.asarray(inputs["mpWu2"], f32)
    bu2 = np.asarray(inputs["mpbu2"], f32)
    Wn1 = np.asarray(inputs["Wn1"], f32)
    bn1 = np.asarray(inputs["bn1"], f32)
    Wn2 = np.asarray(inputs["Wn2"], f32)
    bn2 = np.asarray(inputs["bn2"], f32)
    Wr1 = np.asarray(inputs["Wr1"], f32)                       # [384,128]
    br1 = np.asarray(inputs["br1"], f32)
    Wr2 = np.asarray(inputs["Wr2"], f32)                       # [128,1]
    br2 = np.asarray(inputs["br2"], f32)                       # [1]

    c0 = b1[0] + emb @ (W1[0, 0:128] + W1[0, 128:256])         # layer-0 fold
    embPlusBu2 = emb + bu2[0]

    # wmat: concatenation of [128,128] lhsT tiles (+ the 256-wide iota)
    mats = {}
    mcols = []

    def addm(name, arr):
        mats[name] = sum(a.shape[1] for a in mcols)
        mcols.append(np.ascontiguousarray(arr.astype(f32)))

    for l in range(4):
        addm(f"W1a{l}", W1[l, 0:128])
        addm(f"W1b{l}", W1[l, 128:256])
        addm(f"W1c{l}", W1[l, 256:384])
        addm(f"W2{l}", W2[l])
        addm(f"Wu1{l}", Wu1[l])
        addm(f"Wu2{l}", Wu2[l])
    addm("Wn1", Wn1)
    addm("Wn2", Wn2)
    addm("Wr1a", Wr1[0:128])
    addm("Wr1b", Wr1[128:256])
    addm("Wr1c", Wr1[256:384])
    addm("ident", np.eye(128, dtype=f32))
    addm("iota", np.tile(np.arange(WIN, dtype=f32), (128, 1)))
    wmat = np.concatenate(mcols, axis=1)

    import ml_dtypes
    bf16 = ml_dtypes.bfloat16
    matsb = {}
    bcols = []

    def addb(name, arr):
        matsb[name] = len(bcols) * 128
        bcols.append(np.ascontiguousarray(arr.astype(bf16)))

    for l in range(4):
        addb(f"W1a{l}", W1[l, 0:128])
        addb(f"W1b{l}", W1[l, 128:256])
        addb(f"W1c{l}", W1[l, 256:384])
    wmatb = np.concatenate(bcols, axis=1)

    # wvec: [128, nv] of per-partition vectors
    vecs = {}
    vcols = []

    def addv(name, v):
        vecs[name] = len(vcols)
        vcols.append(np.asarray(v, f32).reshape(128))

    addv("be", be)
    addv("emb", emb)
    addv("c0", c0)
    addv("embPlusBu2", embPlusBu2)
    addv("bn1", bn1)
    addv("bn2", bn2)
    addv("br1", br1)
    addv("Wr2", Wr2[:, 0])
    addv("br2", np.full(128, br2[0], f32))
    for l in range(4):
        addv(f"b1{l}", b1[l])
        addv(f"b2{l}", b2[l])
        addv(f"bs{l}", np.full(128, bs[l, 0], f32))
        addv(f"bu1{l}", bu1[l])
        addv(f"bu2{l}", bu2[l])
        addv(f"Ws{l}", Ws[l, :, 0])
    wvec = np.stack(vcols, axis=1)

    base_map = {
        "wmat": wmat,
        "wmatb": wmatb,
        "wvec": np.ascontiguousarray(wvec),
        "WeT": np.ascontiguousarray(We),               # [3,128]
        "onesRow": np.ones((1, 128), f32),
        "invcntR": invcntR,
        "presentR": presentR,
    }

    # ---- per-core data ----
    in_maps = []
    for c in range(NCORES):
        lo = c * NS
        src_idx = np.zeros(Ecap, np.int64)
        dst_idx = np.zeros(Ecap, np.int64)
        dloc = np.full(Ecap, -1.0, f32)
        d0T = np.zeros((3, Ecap), f32)
        for w in range(NW):
            gw = c * NW + w
            a, b = wstart[gw], wstart[gw + 1]
            k = b - a
            off = w * CAP
            assert k <= CAP
            src_idx[off:off + k] = psrc_s[a:b]
            dst_idx[off:off + k] = pdst_s[a:b]
            dloc[off:off + k] = (pdst_s[a:b] - (lo + w * WIN)).astype(f32)
            d0T[:, off:off + k] = d0_s[a:b].T
        # valid-node mask for this core's slots
        occ = np.zeros(Npad, f32)
        occ[padpos] = 1.0
        vmask = occ[lo:lo + NS].reshape(1, NS)
        Gplace = np.zeros((8, G), f32)
        Gplace[np.arange(8), c * 8 + np.arange(8)] = 1.0
        absentM = np.full((128, G), F32MIN, f32)
        absentM[:, c * 8:(c + 1) * 8] = 0.0

        m = dict(base_map)
        m.update({
            "srcW": _wrap_idxs(src_idx),
            "dstW": _wrap_idxs(dst_idx),
            "dstloc": np.ascontiguousarray(dloc.reshape(C, 128).T),
            "d0T": d0T,
            "vmaskR": vmask,
            "Gplace": Gplace,
            "absentM": absentM,
        })
        in_maps.append(m)

    params = dict(NG=NG, NS=NS, Npad=Npad, NW=NW, KW=KW, CAP=CAP,
                  Ecap=Ecap, C=C, G=G, mats=mats, vecs=vecs, matsb=matsb,
                  wmat_cols=wmat.shape[1], wvec_cols=wvec.shape[1],
                  wmatb_cols=wmatb.shape[1])
    return params, in_maps


def _build(P):
    import concourse.bacc as bacc
    import concourse.mybir as mybir
    import concourse.tile as tile

    f32 = mybir.dt.float32
    bf16 = mybir.dt.bfloat16
    i16 = mybir.dt.int16
    AF = mybir.ActivationFunctionType
    OP = mybir.AluOpType
    RG = [list(range(NCORES))]

    NS, NW, KW, CAP, Ecap, C, G, NG = (P["NS"], P["NW"], P["KW"], P["CAP"],
                                       P["Ecap"], P["C"], P["G"], P["NG"])
    HKW = KW // 2
    HN = HKW * 128           # idxs per gather call
    NSUB = KW // 4           # 512-edge subtiles per window

    import os
    NL = int(os.environ.get("K_NLAYERS", "4"))
    NO_GATHER = os.environ.get("K_NO_GATHER") == "1"
    GATHER_LOCAL = os.environ.get("K_GATHER_LOCAL") == "1"
    NO_AG = os.environ.get("K_NO_AG") == "1"
    nc = bacc.Bacc("TRN2", target_bir_lowering=False, debug=False,
                   num_devices=NCORES)

    t_wmat = nc.dram_tensor("wmat", [128, P["wmat_cols"]], f32, kind="ExternalInput")
    t_wvec = nc.dram_tensor("wvec", [128, P["wvec_cols"]], f32, kind="ExternalInput")
    t_wmatb = nc.dram_tensor("wmatb", [128, P["wmatb_cols"]], bf16, kind="ExternalInput")
    t_We = nc.dram_tensor("WeT", [3, 128], f32, kind="ExternalInput")
    t_ones = nc.dram_tensor("onesRow", [1, 128], f32, kind="ExternalInput")
    t_inv = nc.dram_tensor("invcntR", [128, G], f32, kind="ExternalInput")
    t_pres = nc.dram_tensor("presentR", [128, G], f32, kind="ExternalInput")
    t_srcW = nc.dram_tensor("srcW", [128, Ecap // 16], i16, kind="ExternalInput")
    t_dstW = nc.dram_tensor("dstW", [128, Ecap // 16], i16, kind="ExternalInput")
    t_dstloc = nc.dram_tensor("dstloc", [128, C], f32, kind="ExternalInput")
    t_d0T = nc.dram_tensor("d0T", [3, Ecap], f32, kind="ExternalInput")
    t_vmask = nc.dram_tensor("vmaskR", [1, NS], f32, kind="ExternalInput")
    t_gplace = nc.dram_tensor("Gplace", [8, G], f32, kind="ExternalInput")
    t_absent = nc.dram_tensor("absentM", [128, G], f32, kind="ExternalInput")
    t_out = nc.dram_tensor("out", [G, 1], f32, kind="ExternalOutput")

    with tile.TileContext(nc) as tc:
        with (
            tc.tile_pool(name="sbc", bufs=1) as sbc,
            tc.tile_pool(name="sbp", bufs=1) as sbp,      # persistent feats
            tc.tile_pool(name="sbg", bufs=2) as sbg,      # gather landing
            tc.tile_pool(name="sbw", bufs=2) as sbw,      # working tiles
            tc.tile_pool(name="sbs", bufs=4) as sbs,      # S tiles
            tc.tile_pool(name="ps_mm", bufs=3, space="PSUM") as ps_mm,
            tc.tile_pool(name="ps_me", bufs=1, space="PSUM") as ps_me,
            tc.tile_pool(name="ps_ew", bufs=1, space="PSUM") as ps_ew,
            tc.tile_pool(name="ps_ms", bufs=2, space="PSUM") as ps_ms,
            tc.tile_pool(name="dram", bufs=1, space="DRAM") as dram,
        ):
            # ---- constants ----
            wmat = sbc.tile([128, P["wmat_cols"]], f32, tag="wmat")
            nc.sync.dma_start(wmat[:], t_wmat[:])
            wvec = sbc.tile([128, P["wvec_cols"]], f32, tag="wvec")
            nc.sync.dma_start(wvec[:], t_wvec[:])
            wmatb = sbc.tile([128, P["wmatb_cols"]], bf16, tag="wmatb")
            nc.sync.dma_start(wmatb[:], t_wmatb[:])
            WeT = sbc.tile([3, 128], f32, tag="WeT")
            nc.sync.dma_start(WeT[:], t_We[:])
            onesR = sbc.tile([1, 128], f32, tag="onesR")
            nc.sync.dma_start(onesR[:], t_ones[:])
            invR = sbc.tile([128, G], f32, tag="invR")
            nc.sync.dma_start(invR[:], t_inv[:])
            presR = sbc.tile([128, G], f32, tag="presR")
            nc.sync.dma_start(presR[:], t_pres[:])
            srcW = sbc.tile([128, Ecap // 16], i16, tag="srcW")
            nc.sync.dma_start(srcW[:], t_srcW[:])
            dstW = sbc.tile([128, Ecap // 16], i16, tag="dstW")
            nc.sync.dma_start(dstW[:], t_dstW[:])
            dstloc = sbc.tile([128, C], f32, tag="dstloc")
            nc.sync.dma_start(dstloc[:], t_dstloc[:])
            gplace = sbc.tile([8, G], f32, tag="gplace")
            nc.sync.dma_start(gplace[:], t_gplace[:])
            absentM = sbc.tile([128, G], f32, tag="absentM")
            nc.sync.dma_start(absentM[:], t_absent[:])

            def W(name):
                o = P["mats"][name]
                return wmat[:, o:o + 128]

            def Wb(name):
                o = P["matsb"][name]
                return wmatb[:, o:o + 128]

            def V(name):
                o = P["vecs"][name]
                return wvec[:, o:o + 1]

            ident = W("ident")
            o = P["mats"]["iota"]
            iotaT = wmat[:, o:o + WIN]

            tc.strict_bb_all_engine_barrier()

            # ---- DRAM work buffers ----
            dT = [dram.tile([128, Ecap], bf16, tag=f"dT{i}", name=f"dT{i}") for i in range(2)]
            ag_in = [dram.tile([NS, 128], bf16, tag=f"agin{l}", name=f"agin{l}") for l in range(3)]
            ag_out = [dram.tile([P["Npad"], 128], bf16, addr_space="Shared",
                                tag=f"agout{l}", name=f"agout{l}") for l in range(3)]
            if GATHER_LOCAL:
                ag_loc = [dram.tile([P["Npad"], 128], bf16, tag=f"agloc{l}",
                                    name=f"agloc{l}") for l in range(3)]
            else:
                ag_loc = ag_out

            featA = sbp.tile([128, NS], f32, tag="featA")
            featB = sbp.tile([128, NS], f32, tag="featB")
            featOut = sbp.tile([128, NS], f32, tag="featOut")
            fsum = featA   # dead by readout time; WAR tracked by Tile
            fmx = featB

            # ================= message-passing layers =================
            for l in range(NL):
                featC = [None, featA, featB, featA][l]
                featN = [featA, featB, featA, featB][l]
                dprev = dT[(l + 1) % 2]
                dcur = dT[l % 2]

                for w in range(NW):
                    woff = w * CAP
                    # gathers for this window (layers 1..3)
                    gs, gd = [], []
                    if l > 0 and not NO_GATHER:
                        for half in range(2):
                            cs = (woff + half * HN) // 16
                            gt = sbg.tile([128, 1, HN], bf16, tag="gsrc")
                            nc.gpsimd.dma_gather(
                                gt[:], ag_loc[l - 1][:], srcW[:, cs:cs + HN // 16],
                                HN, HN, 128, transpose=True, single_packet=False)
                            gs.append(gt)
                            gt = sbg.tile([128, 1, HN], bf16, tag="gdst")
                            nc.gpsimd.dma_gather(
                                gt[:], ag_loc[l - 1][:], dstW[:, cs:cs + HN // 16],
                                HN, HN, 128, transpose=True, single_packet=False)
                            gd.append(gt)

                    pmsum = ps_ms.tile([128, WIN], f32, tag="pmsum")

                    for s in range(NSUB):
                        soff = woff + s * 512
                        pm = ps_mm.tile([128, 512], f32, tag="pmm")
                        if l == 0:
                            d0s = sbw.tile([3, 512], f32, tag="d0s")
                            nc.sync.dma_start(d0s[:], t_d0T[:, soff:soff + 512])
                            pdf = ps_mm.tile([128, 512], f32, tag="pmm")
                            nc.tensor.matmul(pdf[:], WeT[:], d0s[:],
                                             start=True, stop=True)
                            dtile = sbw.tile([128, 512], f32, tag="dtile")
                            nc.scalar.activation(dtile[:], pdf[:], AF.Silu,
                                                 bias=V("be"))
                            nc.tensor.matmul(pm[:], W("W1c0"), dtile[:],
                                             start=True, stop=True)
                            h1 = sbw.tile([128, 512], f32, tag="h1")
                            nc.scalar.activation(h1[:], pm[:], AF.Silu,
                                                 bias=V("c0"))
                        else:
                            dtile = sbw.tile([128, 512], bf16, tag="dtile")
                            nc.sync.dma_start(dtile[:], dprev[:, soff:soff + 512])
                            half = (s * 512) // HN
                            hoff = s * 512 - half * HN
                            if NO_GATHER:
                                miS, miD = dtile, dtile
                            else:
                                miS = gs[half][:, 0, hoff:hoff + 512]
                                miD = gd[half][:, 0, hoff:hoff + 512]
                            nc.tensor.matmul(pm[:], Wb(f"W1a{l}"), miS[:],
                                             start=True, stop=False)
                            nc.tensor.matmul(pm[:], Wb(f"W1b{l}"), miD[:],
                                             start=False, stop=False)
                            nc.tensor.matmul(pm[:], Wb(f"W1c{l}"), dtile[:],
                                             start=False, stop=True)
                            h1 = sbw.tile([128, 512], f32, tag="h1")
                            nc.scalar.activation(h1[:], pm[:], AF.Silu,
                                                 bias=V(f"b1{l}"))
                        pm2 = ps_mm.tile([128, 512], f32, tag="pmm")
                        nc.tensor.matmul(pm2[:], W(f"W2{l}"), h1[:],
                                         start=True, stop=True)
                        msgT = sbw.tile([128, 512], f32, tag="msgT")
                        nc.scalar.activation(msgT[:], pm2[:], AF.Silu,
                                             bias=V(f"b2{l}"))
                        if l < NL - 1:
                            dnew = sbw.tile([128, 512], bf16, tag="dnew")
                            nc.vector.tensor_add(dnew[:], msgT[:], dtile[:])
                            nc.sync.dma_start(dcur[:, soff:soff + 512], dnew[:])
                        # soft edge weights (4 chunks -> one sigmoid)
                        pew = ps_ew.tile([128, 4], f32, tag="pew")
                        for k in range(4):
                            nc.tensor.matmul(pew[:, k:k + 1],
                                             msgT[:, k * 128:(k + 1) * 128],
                                             V(f"Ws{l}"), start=True, stop=True)
                        ewS = sbw.tile([128, 4], f32, tag="ewS")
                        nc.scalar.activation(ewS[:], pew[:], AF.Sigmoid,
                                             bias=V(f"bs{l}"))
                        # transpose msg chunks -> [e, h]
                        pme = ps_me.tile([128, 512], f32, tag="pme")
                        for k in range(4):
                            nc.tensor.transpose(pme[:, k * 128:(k + 1) * 128],
                                                msgT[:, k * 128:(k + 1) * 128],
                                                ident)
                        msgE = sbw.tile([128, 512], f32, tag="msgE")
                        nc.scalar.copy(msgE[:], pme[:])
                        # selection matrices + segment-sum matmuls
                        for k in range(4):
                            ci = w * KW + s * 4 + k
                            S = sbs.tile([128, WIN], f32, tag="S")
                            nc.vector.tensor_scalar(
                                S[:], iotaT, dstloc[:, ci:ci + 1],
                                ewS[:, k:k + 1],
                                op0=OP.is_equal, op1=OP.mult)
                            nc.tensor.matmul(pmsum[:],
                                             msgE[:, k * 128:(k + 1) * 128],
                                             S[:],
                                             start=(s * 4 + k == 0),
                                             stop=(s * 4 + k == KW - 1))

                    # ---- window update (node MLP on m_sum) ----
                    wsl = slice(w * WIN, (w + 1) * WIN)
                    x = sbw.tile([128, WIN], f32, tag="xw")
                    if l == 0:
                        nc.vector.tensor_scalar_add(x[:], pmsum[:], V("emb"))
                    else:
                        nc.vector.tensor_add(x[:], pmsum[:], featC[:, wsl])
                    pu = ps_mm.tile([128, WIN], f32, tag="pmm")
                    nc.tensor.matmul(pu[:], W(f"Wu1{l}"), x[:],
                                     start=True, stop=True)
                    u1 = sbw.tile([128, WIN], f32, tag="u1")
                    nc.scalar.activation(u1[:], pu[:], AF.Silu,
                                         bias=V(f"bu1{l}"))
                    ph = ps_mm.tile([128, WIN], f32, tag="pmm")
                    nc.tensor.matmul(ph[:], W(f"Wu2{l}"), u1[:],
                                     start=True, stop=True)
                    if l == 0:
                        nc.vector.tensor_scalar_add(featN[:, wsl], ph[:],
                                                    V("embPlusBu2"))
                    else:
                        hn = sbw.tile([128, WIN], f32, tag="hn")
                        nc.scalar.activation(hn[:], ph[:], AF.Identity,
                                             bias=V(f"bu2{l}"))
                        nc.vector.tensor_add(featN[:, wsl], hn[:],
                                             featC[:, wsl])
                    if l < NL - 1:
                        # write updated slice (node-major) for AllGather
                        pwb = ps_mm.tile([128, WIN], f32, tag="pmm")
                        for k in range(2):
                            nc.tensor.transpose(
                                pwb[:, k * 128:(k + 1) * 128],
                                featN[:, w * WIN + k * 128:w * WIN + (k + 1) * 128],
                                ident)
                        wb = sbw.tile([128, WIN], bf16, tag="wb")
                        nc.scalar.copy(wb[:], pwb[:])
                        for k in range(2):
                            r0 = w * WIN + k * 128
                            nc.sync.dma_start(ag_in[l][r0:r0 + 128, :],
                                              wb[:, k * 128:(k + 1) * 128])
                    else:
                        # final node-wise output MLP
                        pn1 = ps_mm.tile([128, WIN], f32, tag="pmm")
                        nc.tensor.matmul(pn1[:], W("Wn1"), featN[:, wsl],
                                         start=True, stop=True)
                        fo1 = sbw.tile([128, WIN], f32, tag="fo1")
                        nc.scalar.activation(fo1[:], pn1[:], AF.Silu,
                                             bias=V("bn1"))
                        pn2 = ps_mm.tile([128, WIN], f32, tag="pmm")
                        nc.tensor.matmul(pn2[:], W("Wn2"), fo1[:],
                                         start=True, stop=True)
                        nc.scalar.activation(featOut[:, wsl], pn2[:],
                                             AF.Identity, bias=V("bn2"))

                if l < NL - 1 and not NO_AG:
                    nc.gpsimd.collective_compute(
                        "AllGather", mybir.AluOpType.bypass,
                        ins=[ag_in[l][:]], outs=[ag_out[l][:]],
                        replica_groups=RG)
                    if GATHER_LOCAL:
                        nc.sync.dma_start(ag_loc[l][:], ag_out[l][:])

            # ================= readout =================
            for w in range(NW):
                wsl = slice(w * WIN, (w + 1) * WIN)
                vbR = sbw.tile([1, WIN], f32, tag="vbR")
                nc.sync.dma_start(vbR[:], t_vmask[:, wsl])
                pvb = ps_mm.tile([128, WIN], f32, tag="pmm")
                nc.tensor.matmul(pvb[:], onesR[:], vbR[:],
                                 start=True, stop=True)
                vb = sbw.tile([128, WIN], f32, tag="vb")
                nc.scalar.copy(vb[:], pvb[:])
                nc.vector.tensor_mul(fsum[:, wsl], featOut[:, wsl], vb[:])
                negm = sbw.tile([128, WIN], f32, tag="negm")
                nc.vector.tensor_scalar(negm[:], vb[:], 1.0, -F32MIN,
                                        op0=OP.subtract, op1=OP.mult)
                nc.vector.tensor_add(fmx[:, wsl], fsum[:, wsl], negm[:])

            rsum8 = sbw.tile([128, 8], f32, tag="rsum8")
            rmax8 = sbw.tile([128, 8], f32, tag="rmax8")
            AX = mybir.AxisListType.X
            for j in range(8):
                nc.vector.tensor_reduce(rsum8[:, j:j + 1],
                                        fsum[:, j * NG:(j + 1) * NG],
                                        axis=AX, op=OP.add)
                nc.vector.tensor_reduce(rmax8[:, j:j + 1],
                                        fmx[:, j * NG:(j + 1) * NG],
                                        axis=AX, op=OP.max)
            # place into [128, G] global-graph columns
            prt = ps_mm.tile([8, 128], f32, tag="pmm")
            nc.tensor.transpose(prt[:], rsum8[:], ident)
            r8T = sbw.tile([8, 128], f32, tag="r8T")
            nc.scalar.copy(r8T[:], prt[:])
            ppl = ps_mm.tile([128, G], f32, tag="pmm")
            nc.tensor.matmul(ppl[:], r8T[:], gplace[:], start=True, stop=True)
            rsum64 = sbw.tile([128, G], f32, tag="rsum64")
            nc.scalar.copy(rsum64[:], ppl[:])

            prtm = ps_mm.tile([8, 128], f32, tag="pmm")
            nc.tensor.transpose(prtm[:], rmax8[:], ident)
            r8Tm = sbw.tile([8, 128], f32, tag="r8Tm")
            nc.scalar.copy(r8Tm[:], prtm[:])
            pplm = ps_mm.tile([128, G], f32, tag="pmm")
            nc.tensor.matmul(pplm[:], r8Tm[:], gplace[:], start=True, stop=True)
            rmax64 = sbw.tile([128, G], f32, tag="rmax64")
            nc.scalar.copy(rmax64[:], pplm[:])
            rmax64m = sbw.tile([128, G], f32, tag="rmax64m")
            nc.vector.tensor_add(rmax64m[:], rmax64[:], absentM[:])

            # cross-core reduction of per-graph stats
            cc_si = dram.tile([128, G], f32, tag="ccsi")
            cc_so = dram.tile([128, G], f32, addr_space="Shared", tag="ccso")
            cc_mi = dram.tile([128, G], f32, tag="ccmi")
            cc_mo = dram.tile([128, G], f32, addr_space="Shared", tag="ccmo")
            nc.sync.dma_start(cc_si[:], rsum64[:])
            nc.gpsimd.collective_compute("AllReduce", OP.add,
                                         ins=[cc_si[:]], outs=[cc_so[:]],
                                         replica_groups=RG)
            nc.sync.dma_start(cc_mi[:], rmax64m[:])
            nc.gpsimd.collective_compute("AllReduce", OP.max,
                                         ins=[cc_mi[:]], outs=[cc_mo[:]],
                                         replica_groups=RG)
            rsumT = sbw.tile([128, G], f32, tag="rsumT")
            nc.sync.dma_start(rsumT[:], cc_so[:])
            rmaxTr = sbw.tile([128, G], f32, tag="rmaxTr")
            nc.sync.dma_start(rmaxTr[:], cc_mo[:])
            rmaxT = sbw.tile([128, G], f32, tag="rmaxT")
            nc.vector.tensor_mul(rmaxT[:], rmaxTr[:], presR[:])
            rmeanT = sbw.tile([128, G], f32, tag="rmeanT")
            nc.vector.tensor_mul(rmeanT[:], rsumT[:], invR[:])

            # readout MLP
            pq = ps_mm.tile([128, G], f32, tag="pmm")
            nc.tensor.matmul(pq[:], W("Wr1a"), rsumT[:], start=True, stop=False)
            nc.tensor.matmul(pq[:], W("Wr1b"), rmeanT[:], start=False, stop=False)
            nc.tensor.matmul(pq[:], W("Wr1c"), rmaxT[:], start=False, stop=True)
            q = sbw.tile([128, G], f32, tag="q")
            nc.scalar.activation(q[:], pq[:], AF.Relu, bias=V("br1"))
            po = ps_mm.tile([1, G], f32, tag="pmm")
            nc.tensor.matmul(po[:], V("Wr2"), q[:], start=True, stop=True)
            ofin = sbw.tile([1, G], f32, tag="ofin")
            nc.scalar.activation(ofin[:], po[:], AF.Identity,
                                 bias=wvec[0:1, P["vecs"]["br2"]:P["vecs"]["br2"] + 1])
            nc.sync.dma_start(t_out.ap().rearrange("g t -> t g"), ofin[:])

    nc.compile()
    return nc


_CACHE = {}


def kernel(**inputs) -> np.ndarray:
    from concourse.bass_utils import run_bass_kernel_spmd

    import os
    params, in_maps = _prep(inputs)
    key = (params["NS"], params["KW"], os.environ.get("K_NLAYERS", "4"),
           os.environ.get("K_NO_GATHER"), os.environ.get("K_GATHER_LOCAL"),
           os.environ.get("K_NO_AG"))
    if key not in _CACHE:
        _CACHE[key] = _build(params)
    nc = _CACHE[key]
    res = run_bass_kernel_spmd(nc, in_maps, list(range(NCORES)))
    return np.asarray(res.results[0]["out"])

